# revision 24
# baseline (speedup 1.0000x reference)
"""GAT layer kernel for Trainium2, 8 NeuronCores.

Strategy (src-range sharding, no collectives):
  - Host: LPT-balance src nodes over 392 global (core, tile) slots so each
    tile owns ~2048 edges (C=16 chunks of 128).  Table rows live in
    per-core ROTATED tile-slot space: core k's own nodes occupy rows
    [0, ntiles*128) so phase-2 fallback/s data comes straight from SBUF.
  - Phase 1 (device): whaug table [npad, 264] fp16 rows
    [Wh0(128) | Wh1(128) | t0 t1 s0 s1 | pad2] via x @ [W | wt | ws]
    matmuls; PSUM->fp16 conversion on the Activation engine (one copy per
    tile); own-tile rows also kept in SBUF (fb_all) + s-pairs (s_all).
  - Phase 2 (device), per tile:
      Pool : C indirect row-gathers (dst rows, 128 offsets each)
      PE   : srcL broadcast (ones @ srcL_row), C tiny s-expand matmuls,
             C aggregation matmuls (one-hot lhsT)
      DVE  : one-hot builds (oneh [e,slot], oneT [slot,e]), rhs scaling by
             p, tiny e-ops; reciprocal
      ACT  : exp, output scaling
    Softmax uses a global SHIFT (softmax-invariant); deg-0 fallback uses
    out = (num + d*Wh_own) / (den + d) with d=1e-30 (exact for den=0).
"""

import math
import sys
from dataclasses import dataclass

import numpy as np

sys.path.insert(0, "/opt/trn_rl_repo")

import concourse.bass as bass
import concourse.mybir as mybir
import concourse.tile as tile
from concourse import bacc
from concourse.bass import IndirectOffsetOnAxis
from concourse.bass_utils import run_bass_kernel_spmd

N_NODES = 50000
IN_DIM = 256
OUT_DIM = 128
NUM_HEADS = 2

P = 128
AUGW = 260  # table row: Wh0(128), Wh1(128), t0, t1, s0, s1
USEW = 260
RHSW = 258  # aggregation matmul width: 256 features + 2 denominator cols
SHIFT = 4.0
DELTA = 1e-30

F32 = mybir.dt.float32
F16 = mybir.dt.float16
I32 = mybir.dt.int32


@dataclass(frozen=True)
class Cfg:
    n_nodes: int
    n_cores: int
    C: int
    span_tiles: int = 16
    reps: int = 1
    npass: int = 4

    @property
    def nodes_per_core(self):
        return self.n_nodes // self.n_cores

    @property
    def ntiles(self):
        return (self.nodes_per_core + P - 1) // P

    @property
    def npad(self):
        return self.n_cores * self.ntiles * P


def _ap_expand(ap, dims):
    """Return an AP keeping ap's partition dim and replacing the free dims
    with `dims` = list of (step, count) pairs (element units)."""
    return bass.AP(ap.tensor, ap.offset, [list(ap.ap[0])] + [[s, c] for s, c in dims])


def host_prep(x, edge_index, W_w, W_b, a, n_cores=8):
    """Index/layout preprocessing + parameter folding."""
    x = np.asarray(x, dtype=np.float32)
    edge_index = np.asarray(edge_index)
    W_w = np.asarray(W_w, dtype=np.float32)
    W_b = np.asarray(W_b, dtype=np.float32)
    a = np.asarray(a, dtype=np.float32)
    assert np.abs(W_b).max() == 0.0, "nonzero bias not supported"

    n_nodes, in_dim = x.shape
    D = OUT_DIM
    n_edges = edge_index.shape[1]

    # wbig columns: [W (256) | wt0 wt1 | ws0 ws1]  ->  ps = [Wh0 Wh1 t0 t1 s0 s1]
    a_src, a_dst = a[:D], a[D:]
    ws0 = W_w[:, 0:D] @ a_src
    ws1 = W_w[:, D : 2 * D] @ a_src
    wt0 = W_w[:, 0:D] @ a_dst
    wt1 = W_w[:, D : 2 * D] @ a_dst
    wbig = np.concatenate(
        [W_w, wt0[:, None], wt1[:, None], ws0[:, None], ws1[:, None]], axis=1
    ).astype(np.float16)  # [in_dim, 260]

    src = np.asarray(edge_index[0], dtype=np.int64)
    dst = np.asarray(edge_index[1], dtype=np.int64)
    order = np.argsort(src, kind="stable")
    src_s = src[order]
    dst_s = dst[order].astype(np.int64)

    npc = n_nodes // n_cores
    ntiles = (npc + P - 1) // P

    # LPT: assign nodes to (core,tile,slot), balancing per-tile edge counts.
    import heapq

    ntile_tot = n_cores * ntiles
    deg_all = np.bincount(src, minlength=n_nodes)
    order_n = np.argsort(-deg_all, kind="stable")
    heap = [(0, t) for t in range(ntile_tot)]
    heapq.heapify(heap)
    fill = np.zeros(ntile_tot, dtype=np.int64)
    node_tile = np.zeros(n_nodes, dtype=np.int64)
    node_slot = np.zeros(n_nodes, dtype=np.int64)
    for n in order_n:
        while True:
            w, t = heapq.heappop(heap)
            if fill[t] < P:
                break
        node_tile[n] = t
        node_slot[n] = fill[t]
        fill[t] += 1
        if fill[t] < P:
            heapq.heappush(heap, (w + int(deg_all[n]), t))

    grow = node_tile * P + node_slot  # global tile-slot row of each node

    gtile = node_tile[src_s]
    pos = node_slot[src_s]
    # sort by (tile, rotated dst-row of the owning core) so chunk c of a
    # tile covers a dst-row prefix bound in that core's table write order
    # -> gathers can start before the whole table is written
    rows_pc_ = ntiles * P
    npad_ = n_cores * rows_pc_
    rot_key = (grow[dst_s] - (gtile // ntiles) * rows_pc_) % npad_
    order2 = np.lexsort((rot_key, gtile))
    gtile = gtile[order2]
    dst_s = dst_s[order2]
    pos = pos[order2]

    counts = np.bincount(gtile, minlength=ntile_tot)
    C = int(math.ceil(counts.max() / P))
    cfg = Cfg(n_nodes=n_nodes, n_cores=n_cores, C=C)
    npad = cfg.npad
    slots_per_tile = C * P

    # xT in global tile-slot space (empty slots -> 0 rows)
    xTslot = np.zeros((in_dim, npad), dtype=np.float16)
    xTslot[:, grow] = x.T.astype(np.float16)

    starts = np.zeros(ntile_tot, dtype=np.int64)
    starts[1:] = np.cumsum(counts)[:-1]
    slot_in_tile = np.arange(n_edges) - starts[gtile]

    # per-edge-slot arrays in (tile, chunk, partition) space
    dstG = np.full((ntile_tot, slots_per_tile), -1, dtype=np.int64)  # dst grow
    srcL = np.full((ntile_tot, slots_per_tile), 255, dtype=np.uint8)
    flat = gtile * slots_per_tile + slot_in_tile
    dstG.reshape(-1)[flat] = grow[dst_s]
    srcL.reshape(-1)[flat] = pos.astype(np.uint8)

    def to_core_layout(arr, fillval):
        # [tiles, C*P] -> per-core [P, ntiles*C]
        a4 = arr.reshape(n_cores, ntiles, C, P)
        return np.ascontiguousarray(np.transpose(a4, (0, 3, 1, 2))).reshape(
            n_cores, P, ntiles * C
        )

    dstG_c = to_core_layout(dstG, 0)
    srcL_c = to_core_layout(srcL, -1.0)

    # Rtab[t, c] = 1 + max over cores of the rotated dst row in chunk c of
    # tile t (pad slots have dstG=0 -> rotated row npad-shift; make pads
    # point at row 0 instead so they don't inflate the bound)

    iota16 = np.broadcast_to(
        np.arange(P, dtype=np.uint8), (P, P)
    ).copy()  # iota16[p, j] = j
    iota_col = np.arange(P, dtype=np.uint8).reshape(P, 1).copy()

    shared = {"wbig": wbig, "iota16": iota16, "iotac": iota_col}
    shared["_Rtab"] = None  # placeholder, set below
    per_core = []
    rows_pc = ntiles * P
    # srcL row layout for the PE broadcast: [1, ntiles*C*P]
    Rtab = np.zeros((ntiles, C), dtype=np.int64)
    dstI_all = []
    for k in range(n_cores):
        shift = k * rows_pc
        rot = (dstG_c[k] - shift) % npad
        rot[dstG_c[k] < 0] = 0  # pad slots -> row 0
        dstI_all.append(rot.astype(np.int32))
        r3 = rot.reshape(P, ntiles, C)
        Rtab = np.maximum(Rtab, r3.max(axis=0))
    Rtab = Rtab + 1
    for k in range(n_cores):
        shift = k * rows_pc
        xT_k = np.roll(xTslot, -shift, axis=1)
        dstI_k = dstI_all[k]
        srcL_k = srcL_c[k]
        # row layout value at (t, c, p) = srcL_k[p, t*C + c]
        srcR_k = np.ascontiguousarray(
            np.transpose(srcL_k.reshape(P, ntiles, C), (1, 2, 0))
        ).reshape(1, ntiles * C * P)
        mine = (node_tile >= k * ntiles) & (node_tile < (k + 1) * ntiles)
        nodes_k = np.nonzero(mine)[0]
        rows_k = (node_tile[nodes_k] - k * ntiles) * P + node_slot[nodes_k]
        per_core.append(
            {
                "xT": xT_k,
                "dstI": dstI_k,
                "srcL": srcL_k,
                "srcR": srcR_k,
                "_nodes": nodes_k,
                "_rows": rows_k,
            }
        )
    shared["_Rtab"] = Rtab
    return cfg, shared, per_core


def build_program(cfg: Cfg, rtab, marks=None):
    """rtab: [ntiles, C] int array; rtab[t][c] = exclusive upper bound on
    table rows referenced by chunk c of tile t (edges are dst-sorted within
    each tile, so this is a prefix bound enabling gather/build overlap)."""
    C, ntiles, npad = cfg.C, cfg.ntiles, cfg.npad
    NP_ = cfg.npass
    CP = (C + NP_ - 1) // NP_
    bounds = [min(C, i * CP) for i in range(NP_ + 1)]
    CPmax = max(bounds[i + 1] - bounds[i] for i in range(NP_))

    nc = bacc.Bacc("TRN2", target_bir_lowering=False, debug=False)

    def _mark(label):
        if marks is not None:
            marks[label] = sum(len(b.instructions) for b in nc.m.functions[0].blocks)

    U8 = mybir.dt.uint8

    xT_d = nc.dram_tensor("xT", [IN_DIM, npad], F16, kind="ExternalInput")
    wbig_d = nc.dram_tensor("wbig", [IN_DIM, USEW], F16, kind="ExternalInput")
    iota16_d = nc.dram_tensor("iota16", [P, P], U8, kind="ExternalInput")
    iotac_d = nc.dram_tensor("iotac", [P, 1], U8, kind="ExternalInput")
    dstI_d = nc.dram_tensor("dstI", [P, ntiles * C], I32, kind="ExternalInput")
    srcL_d = nc.dram_tensor("srcL", [P, ntiles * C], U8, kind="ExternalInput")
    srcR_d = nc.dram_tensor("srcR", [1, ntiles * C * P], U8, kind="ExternalInput")
    out_d = nc.dram_tensor("out", [ntiles * P, 2 * OUT_DIM], F32, kind="ExternalOutput")

    whaug_d = nc.dram_tensor("whaug", [npad, AUGW], F16)
    whaug_ref = whaug_d[:, :]

    n_alltiles = npad // P

    with tile.TileContext(nc) as tc:
        with (
            tc.tile_pool(name="const", bufs=1) as constp,
            tc.tile_pool(name="xk", bufs=2) as xkp,
            tc.tile_pool(name="bld_ps", bufs=2, space="PSUM") as bldps,
            tc.tile_pool(name="augg", bufs=2) as auggp,
            tc.tile_pool(name="own", bufs=1) as ownp,
            tc.tile_pool(name="gall", bufs=6) as gallp,
            tc.tile_pool(name="srcr", bufs=2) as srcrp,
            tc.tile_pool(name="oneh", bufs=4) as onehp,
            tc.tile_pool(name="rhs", bufs=4) as rhsp,
            tc.tile_pool(name="s_ps", bufs=2, space="PSUM") as spsp,
            tc.tile_pool(name="agg_ps", bufs=3, space="PSUM") as aggps,
            tc.tile_pool(name="fin", bufs=4) as finp,
            tc.tile_pool(name="og", bufs=2) as ogp,
        ):
            # ---------------- constants ----------------
            wb = constp.tile([P, 2, USEW], F16, tag="wb")
            nc.sync.dma_start(
                out=wb[:], in_=wbig_d[:, :].rearrange("(kt kp) c -> kp kt c", kp=P)
            )
            iota16_t = constp.tile([P, P], U8, tag="iota16")
            iotac_t = constp.tile([P, 1], U8, tag="iotac")
            dstI_t = constp.tile([P, ntiles * C], I32, tag="dstI")
            srcL_t = constp.tile([P, ntiles * C], U8, tag="srcL")
            shift_t = constp.tile([P, 1], F32, tag="shift")
            nc.vector.memset(shift_t[:], -SHIFT)

            def load_p2_consts():
                nc.sync.dma_start(out=iota16_t[:], in_=iota16_d[:, :])
                nc.sync.dma_start(out=iotac_t[:], in_=iotac_d[:, :])
                nc.sync.dma_start(out=dstI_t[:], in_=dstI_d[:, :])
                nc.sync.dma_start(out=srcL_t[:], in_=srcL_d[:, :])
            fb_all = ownp.tile([P, ntiles, 2 * OUT_DIM], F16, tag="fb_all")
            s_all = ownp.tile([P, ntiles, 2], F16, tag="s_all")
            s16_all = ownp.tile([P, ntiles, C, 2], F16, tag="s16_all")
            agg_sb = ownp.tile([P, ntiles, RHSW], F32, tag="agg_sb")

            _mark("consts_end")

            # ---------------- phase 1: build whaug table ----------------
            def s_expand(t):
                srcr = srcrp.tile([P, C, P], U8, tag="srcr")
                sl = srcR_d[0:1, t * C * P : (t + 1) * C * P]
                nc.sync.dma_start(
                    out=srcr[:],
                    in_=bass.AP(sl.tensor, sl.offset, [[0, P], [1, C * P]]),
                )
                oneT = onehp.tile([P, C, P], F16, tag="oneT")
                nc.vector.tensor_tensor(
                    out=oneT[:],
                    in0=_ap_expand(iotac_t[:], [(0, C), (0, P)]),
                    in1=srcr[:],
                    op=mybir.AluOpType.is_equal,
                )
                s_ps = spsp.tile([P, C, 2], F32, tag="s_ps")
                for c in range(C):
                    nc.tensor.matmul(
                        out=s_ps[:, c, :],
                        lhsT=oneT[:, c, :],
                        rhs=s_all[:, t, :],
                        start=True,
                        stop=True,
                    )
                nc.vector.tensor_copy(out=s16_all[:, t, :, :], in_=s_ps[:])

            n0 = 0
            while n0 < n_alltiles:
                span = min(cfg.span_tiles, n_alltiles - n0)
                xk = xkp.tile([P, 2, cfg.span_tiles * P], F16, tag="xk")
                for kt in range(2):
                    nc.sync.dma_start(
                        out=xk[:, kt, 0 : span * P],
                        in_=xT_d[kt * P : (kt + 1) * P, n0 * P : (n0 + span) * P],
                    )
                g0 = 0
                while g0 < span:
                    GRP = 2 if n0 < 64 else 4
                    grp = min(GRP, span - g0)
                    aug = auggp.tile([P, 4, AUGW], F16, tag="aug")
                    for g in range(grp):
                        nt = g0 + g
                        gtile = n0 + nt
                        ps = bldps.tile([P, USEW], F32, tag="bld")
                        for kt in range(2):
                            nc.tensor.matmul(
                                out=ps[:],
                                lhsT=xk[:, kt, nt * P : (nt + 1) * P],
                                rhs=wb[:, kt, :],
                                start=(kt == 0),
                                stop=(kt == 1),
                            )
                        nc.scalar.activation(
                            out=aug[:, g, 0:USEW],
                            in_=ps[:],
                            func=mybir.ActivationFunctionType.Copy,
                        )
                        if gtile < ntiles:
                            nc.vector.tensor_copy(
                                out=fb_all[:, gtile, :], in_=ps[:, 0 : 2 * OUT_DIM]
                            )
                            nc.vector.tensor_copy(
                                out=s_all[:, gtile, :], in_=ps[:, 258:260]
                            )
                    r0 = (n0 + g0) * P
                    nc.sync.dma_start(
                        out=whaug_d[r0 : r0 + grp * P, :].rearrange(
                            "(g p) c -> p g c", p=P
                        ),
                        in_=aug[:, 0:grp, :],
                    )
                    g0 += grp
                n0 += span
                if n0 == cfg.span_tiles:
                    load_p2_consts()

            _mark("p1_end")

            for t in range(ntiles):
                s_expand(t)

            # ---------------- phase 2: passes of CP chunks ----------------
            OGRP = 8

            def compute(t, p):
                c0 = bounds[p]
                c1 = bounds[p + 1]
                nch = c1 - c0
                gall = gallp.tile([P, CPmax, USEW], F16, tag="gall")
                for c in range(c0, c1):
                    R = int(rtab[t][c])
                    bound = bass.AP(
                        whaug_ref.tensor, 0, [[AUGW, R], [1, USEW]]
                    )
                    nc.gpsimd.indirect_dma_start(
                        out=gall[:, c - c0, :],
                        out_offset=None,
                        in_=bound,
                        in_offset=IndirectOffsetOnAxis(
                            ap=dstI_t[:, t * C + c : t * C + c + 1], axis=0
                        ),
                    )
                # e = lrelu(s + t); pexp = exp(e - SHIFT)
                e_t = finp.tile([P, CPmax * 2], F32, tag="e_t")
                nc.vector.tensor_tensor(
                    out=e_t[:, 0 : nch * 2],
                    in0=s16_all[:, t, c0:c1, :],
                    in1=gall[:, 0:nch, 256:258],
                    op=mybir.AluOpType.add,
                )
                e_s = finp.tile([P, CPmax * 2], F32, tag="e_s")
                nc.vector.tensor_scalar(
                    out=e_s[:, 0 : nch * 2], in0=e_t[:, 0 : nch * 2],
                    scalar1=0.2, scalar2=None, op0=mybir.AluOpType.mult,
                )
                lr_t = finp.tile([P, CPmax * 2], F32, tag="lr_t")
                nc.vector.tensor_tensor(
                    out=lr_t[:, 0 : nch * 2], in0=e_t[:, 0 : nch * 2],
                    in1=e_s[:, 0 : nch * 2], op=mybir.AluOpType.max,
                )
                p16 = finp.tile([P, CPmax, 2], F16, tag="p16")
                nc.scalar.activation(
                    out=p16[:, 0:nch, :].rearrange("p c h -> p (c h)"),
                    in_=lr_t[:, 0 : nch * 2],
                    func=mybir.ActivationFunctionType.Exp,
                    bias=shift_t[:, 0:1],
                )
                oneh = onehp.tile([P, CPmax, P], F16, tag="oneh")
                nc.vector.tensor_tensor(
                    out=oneh[:, 0:nch, :],
                    in0=_ap_expand(srcL_t[:, t * C + c0 : t * C + c1], [(1, nch), (0, P)]),
                    in1=_ap_expand(iota16_t[:], [(0, nch), (1, P)]),
                    op=mybir.AluOpType.is_equal,
                )
                rhs = rhsp.tile([P, CPmax, RHSW], F16, tag="rhs")
                nc.vector.tensor_tensor(
                    out=rhs[:, 0:nch, 0 : 2 * OUT_DIM],
                    in0=gall[:, 0:nch, 0 : 2 * OUT_DIM],
                    in1=_ap_expand(p16[:], [(2, nch), (1, 2), (0, OUT_DIM)]),
                    op=mybir.AluOpType.mult,
                )
                nc.vector.tensor_copy(
                    out=rhs[:, 0:nch, 2 * OUT_DIM : RHSW], in_=p16[:, 0:nch, :]
                )
                ps = aggps.tile([P, RHSW], F32, tag="agg")
                for c in range(nch):
                    nc.tensor.matmul(
                        out=ps[:],
                        lhsT=oneh[:, c, :],
                        rhs=rhs[:, c, :],
                        start=(c == 0),
                        stop=(c == nch - 1),
                    )
                if p == 0:
                    nc.vector.tensor_copy(out=agg_sb[:, t, :], in_=ps[:])
                else:
                    nc.vector.tensor_tensor(
                        out=agg_sb[:, t, :], in0=agg_sb[:, t, :], in1=ps[:],
                        op=mybir.AluOpType.add,
                    )

            def finalize(t, og, g):
                den2 = finp.tile([P, 2], F32, tag="den2")
                nc.vector.tensor_scalar(
                    out=den2[:], in0=agg_sb[:, t, 2 * OUT_DIM : RHSW],
                    scalar1=DELTA, scalar2=None, op0=mybir.AluOpType.add,
                )
                rcp = finp.tile([P, 2], F32, tag="rcp")
                nc.vector.reciprocal(out=rcp[:], in_=den2[:])
                num2 = finp.tile([P, 2 * OUT_DIM], F32, tag="num2")
                nc.vector.scalar_tensor_tensor(
                    out=num2[:],
                    in0=fb_all[:, t, :],
                    scalar=DELTA,
                    in1=agg_sb[:, t, 0 : 2 * OUT_DIM],
                    op0=mybir.AluOpType.mult,
                    op1=mybir.AluOpType.add,
                )
                for h in range(2):
                    nc.scalar.activation(
                        out=og[:, g, h * OUT_DIM : (h + 1) * OUT_DIM],
                        in_=num2[:, h * OUT_DIM : (h + 1) * OUT_DIM],
                        func=mybir.ActivationFunctionType.Copy,
                        scale=rcp[:, h : h + 1],
                    )

            for rep in range(cfg.reps):
                for p in range(NP_ - 1):
                    for t in range(ntiles):
                        compute(t, p)
                # last pass: finalize tile t-1 after computing tile t
                og = None
                pend = None

                def flush(tf, og):
                    g = tf % OGRP
                    if g == OGRP - 1 or tf == ntiles - 1:
                        t0 = tf - g
                        nc.sync.dma_start(
                            out=out_d[t0 * P : (tf + 1) * P, :].rearrange(
                                "(g p) c -> p g c", p=P
                            ),
                            in_=og[:, 0 : g + 1, :],
                        )

                for t in range(ntiles):
                    compute(t, NP_ - 1)
                    if pend is not None:
                        g = pend % OGRP
                        if g == 0:
                            og = ogp.tile([P, OGRP, 2 * OUT_DIM], F32, tag="og")
                        finalize(pend, og, g)
                        flush(pend, og)
                    pend = t
                g = pend % OGRP
                if g == 0:
                    og = ogp.tile([P, OGRP, 2 * OUT_DIM], F32, tag="og")
                finalize(pend, og, g)
                flush(pend, og)
            _mark("p2_end")

    nc.compile()
    return nc


_prog_cache = {}


def kernel(x, edge_index, W_w, W_b, a):
    cfg, shared, per_core = host_prep(x, edge_index, W_w, W_b, a, n_cores=8)
    rtab = shared["_Rtab"]
    key = (cfg, rtab.tobytes())
    if key not in _prog_cache:
        _prog_cache[key] = build_program(cfg, rtab)
    nc = _prog_cache[key]
    in_maps = [
        {kk: v for kk, v in {**shared, **pc}.items() if not kk.startswith("_")}
        for pc in per_core
    ]
    res = run_bass_kernel_spmd(nc, in_maps, list(range(cfg.n_cores)))
    out = np.zeros((cfg.n_nodes, 2 * OUT_DIM), dtype=np.float32)
    for k in range(cfg.n_cores):
        pc = per_core[k]
        out[pc["_nodes"]] = res.results[k]["out"][pc["_rows"]]
    return out


# revision 27
# speedup vs baseline: 1.1413x; 1.1413x over previous
"""GAT layer kernel for Trainium2, 8 NeuronCores.

Strategy (src-range sharding, no collectives):
  - Host: LPT-balance src nodes over 392 global (core, tile) slots so each
    tile owns ~2048 edges (C=16 chunks of 128).  Table rows live in
    per-core ROTATED tile-slot space: core k's own nodes occupy rows
    [0, ntiles*128) so phase-2 fallback/s data comes straight from SBUF.
  - Phase 1 (device): whaug table [npad, 264] fp16 rows
    [Wh0(128) | Wh1(128) | t0 t1 s0 s1 | pad2] via x @ [W | wt | ws]
    matmuls; PSUM->fp16 conversion on the Activation engine (one copy per
    tile); own-tile rows also kept in SBUF (fb_all) + s-pairs (s_all).
  - Phase 2 (device), per tile:
      Pool : C indirect row-gathers (dst rows, 128 offsets each)
      PE   : srcL broadcast (ones @ srcL_row), C tiny s-expand matmuls,
             C aggregation matmuls (one-hot lhsT)
      DVE  : one-hot builds (oneh [e,slot], oneT [slot,e]), rhs scaling by
             p, tiny e-ops; reciprocal
      ACT  : exp, output scaling
    Softmax uses a global SHIFT (softmax-invariant); deg-0 fallback uses
    out = (num + d*Wh_own) / (den + d) with d=1e-30 (exact for den=0).
"""

import math
import sys
from dataclasses import dataclass

import numpy as np

sys.path.insert(0, "/opt/trn_rl_repo")

import concourse.bass as bass
import concourse.mybir as mybir
import concourse.tile as tile
from concourse import bacc
from concourse.bass import IndirectOffsetOnAxis
from concourse.bass_utils import run_bass_kernel_spmd

N_NODES = 50000
IN_DIM = 256
OUT_DIM = 128
NUM_HEADS = 2

P = 128
AUGW = 260  # table row: Wh0(128), Wh1(128), t0, t1, s0, s1
USEW = 260
RHSW = 258  # aggregation matmul width: 256 features + 2 denominator cols
SHIFT = 4.0
DELTA = 1e-30

F32 = mybir.dt.float32
F16 = mybir.dt.float16
I32 = mybir.dt.int32


@dataclass(frozen=True)
class Cfg:
    n_nodes: int
    n_cores: int
    C: int
    span_tiles: int = 8
    reps: int = 1
    npass: int = 4

    @property
    def nodes_per_core(self):
        return self.n_nodes // self.n_cores

    @property
    def ntiles(self):
        return (self.nodes_per_core + P - 1) // P

    @property
    def npad(self):
        return self.n_cores * self.ntiles * P


def _ap_expand(ap, dims):
    """Return an AP keeping ap's partition dim and replacing the free dims
    with `dims` = list of (step, count) pairs (element units)."""
    return bass.AP(ap.tensor, ap.offset, [list(ap.ap[0])] + [[s, c] for s, c in dims])


def host_prep(x, edge_index, W_w, W_b, a, n_cores=8):
    """Index/layout preprocessing + parameter folding."""
    x = np.asarray(x, dtype=np.float32)
    edge_index = np.asarray(edge_index)
    W_w = np.asarray(W_w, dtype=np.float32)
    W_b = np.asarray(W_b, dtype=np.float32)
    a = np.asarray(a, dtype=np.float32)
    assert np.abs(W_b).max() == 0.0, "nonzero bias not supported"

    n_nodes, in_dim = x.shape
    D = OUT_DIM
    n_edges = edge_index.shape[1]

    # wbig columns: [W (256) | wt0 wt1 | ws0 ws1]  ->  ps = [Wh0 Wh1 t0 t1 s0 s1]
    a_src, a_dst = a[:D], a[D:]
    ws0 = W_w[:, 0:D] @ a_src
    ws1 = W_w[:, D : 2 * D] @ a_src
    wt0 = W_w[:, 0:D] @ a_dst
    wt1 = W_w[:, D : 2 * D] @ a_dst
    wbig = np.concatenate(
        [W_w, wt0[:, None], wt1[:, None], ws0[:, None], ws1[:, None]], axis=1
    ).astype(np.float16)  # [in_dim, 260]

    src = np.asarray(edge_index[0], dtype=np.int64)
    dst = np.asarray(edge_index[1], dtype=np.int64)
    order = np.argsort(src, kind="stable")
    src_s = src[order]
    dst_s = dst[order].astype(np.int64)

    npc = n_nodes // n_cores
    ntiles = (npc + P - 1) // P

    # LPT: assign nodes to (core,tile,slot), balancing per-tile edge counts.
    import heapq

    ntile_tot = n_cores * ntiles
    deg_all = np.bincount(src, minlength=n_nodes)
    order_n = np.argsort(-deg_all, kind="stable")
    heap = [(0, t) for t in range(ntile_tot)]
    heapq.heapify(heap)
    fill = np.zeros(ntile_tot, dtype=np.int64)
    node_tile = np.zeros(n_nodes, dtype=np.int64)
    node_slot = np.zeros(n_nodes, dtype=np.int64)
    for n in order_n:
        while True:
            w, t = heapq.heappop(heap)
            if fill[t] < P:
                break
        node_tile[n] = t
        node_slot[n] = fill[t]
        fill[t] += 1
        if fill[t] < P:
            heapq.heappush(heap, (w + int(deg_all[n]), t))

    grow = node_tile * P + node_slot  # global tile-slot row of each node

    gtile = node_tile[src_s]
    pos = node_slot[src_s]
    # sort by (tile, rotated dst-row of the owning core) so chunk c of a
    # tile covers a dst-row prefix bound in that core's table write order
    # -> gathers can start before the whole table is written
    rows_pc_ = ntiles * P
    npad_ = n_cores * rows_pc_
    rot_key = (grow[dst_s] - (gtile // ntiles) * rows_pc_) % npad_
    order2 = np.lexsort((rot_key, gtile))
    gtile = gtile[order2]
    dst_s = dst_s[order2]
    pos = pos[order2]

    counts = np.bincount(gtile, minlength=ntile_tot)
    C = int(math.ceil(counts.max() / P))
    cfg = Cfg(n_nodes=n_nodes, n_cores=n_cores, C=C)
    npad = cfg.npad
    slots_per_tile = C * P

    # xT in global tile-slot space (empty slots -> 0 rows)
    xTslot = np.zeros((in_dim, npad), dtype=np.float16)
    xTslot[:, grow] = x.T.astype(np.float16)

    starts = np.zeros(ntile_tot, dtype=np.int64)
    starts[1:] = np.cumsum(counts)[:-1]
    slot_in_tile = np.arange(n_edges) - starts[gtile]

    # per-edge-slot arrays in (tile, chunk, partition) space
    dstG = np.full((ntile_tot, slots_per_tile), -1, dtype=np.int64)  # dst grow
    srcL = np.full((ntile_tot, slots_per_tile), 255, dtype=np.uint8)
    flat = gtile * slots_per_tile + slot_in_tile
    dstG.reshape(-1)[flat] = grow[dst_s]
    srcL.reshape(-1)[flat] = pos.astype(np.uint8)

    def to_core_layout(arr, fillval):
        # [tiles, C*P] -> per-core [P, ntiles*C]
        a4 = arr.reshape(n_cores, ntiles, C, P)
        return np.ascontiguousarray(np.transpose(a4, (0, 3, 1, 2))).reshape(
            n_cores, P, ntiles * C
        )

    dstG_c = to_core_layout(dstG, 0)
    srcL_c = to_core_layout(srcL, -1.0)

    # Rtab[t, c] = 1 + max over cores of the rotated dst row in chunk c of
    # tile t (pad slots have dstG=0 -> rotated row npad-shift; make pads
    # point at row 0 instead so they don't inflate the bound)

    iota16 = np.broadcast_to(
        np.arange(P, dtype=np.uint8), (P, P)
    ).copy()  # iota16[p, j] = j
    iota_col = np.arange(P, dtype=np.uint8).reshape(P, 1).copy()

    shared = {"wbig": wbig, "iota16": iota16, "iotac": iota_col}
    shared["_Rtab"] = None  # placeholder, set below
    per_core = []
    rows_pc = ntiles * P
    # srcL row layout for the PE broadcast: [1, ntiles*C*P]
    Rtab = np.zeros((ntiles, C), dtype=np.int64)
    dstI_all = []
    for k in range(n_cores):
        shift = k * rows_pc
        rot = (dstG_c[k] - shift) % npad
        rot[dstG_c[k] < 0] = 0  # pad slots -> row 0
        dstI_all.append(rot.astype(np.int32))
        r3 = rot.reshape(P, ntiles, C)
        Rtab = np.maximum(Rtab, r3.max(axis=0))
    Rtab = Rtab + 1
    for k in range(n_cores):
        shift = k * rows_pc
        xT_k = np.roll(xTslot, -shift, axis=1)
        dstI_k = dstI_all[k]
        srcL_k = srcL_c[k]
        # row layout value at (t, c, p) = srcL_k[p, t*C + c]
        srcR_k = np.ascontiguousarray(
            np.transpose(srcL_k.reshape(P, ntiles, C), (1, 2, 0))
        ).reshape(1, ntiles * C * P)
        mine = (node_tile >= k * ntiles) & (node_tile < (k + 1) * ntiles)
        nodes_k = np.nonzero(mine)[0]
        rows_k = (node_tile[nodes_k] - k * ntiles) * P + node_slot[nodes_k]
        per_core.append(
            {
                "xT": xT_k,
                "dstI": dstI_k,
                "srcL": srcL_k,
                "srcR": srcR_k,
                "_nodes": nodes_k,
                "_rows": rows_k,
            }
        )
    shared["_Rtab"] = Rtab
    return cfg, shared, per_core


def build_program(cfg: Cfg, rtab, marks=None):
    """rtab: [ntiles, C] int array; rtab[t][c] = exclusive upper bound on
    table rows referenced by chunk c of tile t (edges are dst-sorted within
    each tile, so this is a prefix bound enabling gather/build overlap)."""
    C, ntiles, npad = cfg.C, cfg.ntiles, cfg.npad
    NP_ = cfg.npass
    CP = (C + NP_ - 1) // NP_
    bounds = [min(C, i * CP) for i in range(NP_ + 1)]
    CPmax = max(bounds[i + 1] - bounds[i] for i in range(NP_))

    nc = bacc.Bacc("TRN2", target_bir_lowering=False, debug=False)

    def _mark(label):
        if marks is not None:
            marks[label] = sum(len(b.instructions) for b in nc.m.functions[0].blocks)

    U8 = mybir.dt.uint8

    xT_d = nc.dram_tensor("xT", [IN_DIM, npad], F16, kind="ExternalInput")
    wbig_d = nc.dram_tensor("wbig", [IN_DIM, USEW], F16, kind="ExternalInput")
    iota16_d = nc.dram_tensor("iota16", [P, P], U8, kind="ExternalInput")
    iotac_d = nc.dram_tensor("iotac", [P, 1], U8, kind="ExternalInput")
    dstI_d = nc.dram_tensor("dstI", [P, ntiles * C], I32, kind="ExternalInput")
    srcL_d = nc.dram_tensor("srcL", [P, ntiles * C], U8, kind="ExternalInput")
    srcR_d = nc.dram_tensor("srcR", [1, ntiles * C * P], U8, kind="ExternalInput")
    out_d = nc.dram_tensor("out", [ntiles * P, 2 * OUT_DIM], F32, kind="ExternalOutput")

    whaug_d = nc.dram_tensor("whaug", [npad, AUGW], F16)
    whaug_ref = whaug_d[:, :]

    n_alltiles = npad // P

    with tile.TileContext(nc) as tc:
        with (
            tc.tile_pool(name="const", bufs=1) as constp,
            tc.tile_pool(name="xk", bufs=2) as xkp,
            tc.tile_pool(name="bld_ps", bufs=3, space="PSUM") as bldps,
            tc.tile_pool(name="augg", bufs=2) as auggp,
            tc.tile_pool(name="own", bufs=1) as ownp,
            tc.tile_pool(name="gall", bufs=6) as gallp,
            tc.tile_pool(name="srcr", bufs=2) as srcrp,
            tc.tile_pool(name="oneh", bufs=4) as onehp,
            tc.tile_pool(name="rhs", bufs=4) as rhsp,
            tc.tile_pool(name="s_ps", bufs=2, space="PSUM") as spsp,
            tc.tile_pool(name="agg_ps", bufs=3, space="PSUM") as aggps,
            tc.tile_pool(name="fin", bufs=4) as finp,
            tc.tile_pool(name="og", bufs=2) as ogp,
        ):
            # ---------------- constants ----------------
            wb = constp.tile([P, 2, USEW], F16, tag="wb")
            nc.sync.dma_start(
                out=wb[:], in_=wbig_d[:, :].rearrange("(kt kp) c -> kp kt c", kp=P)
            )
            iota16_t = constp.tile([P, P], U8, tag="iota16")
            iotac_t = constp.tile([P, 1], U8, tag="iotac")
            dstI_t = constp.tile([P, ntiles * C], I32, tag="dstI")
            srcL_t = constp.tile([P, ntiles * C], U8, tag="srcL")
            shift_t = constp.tile([P, 1], F32, tag="shift")
            nc.vector.memset(shift_t[:], -SHIFT)

            def load_p2_consts():
                nc.sync.dma_start(out=iota16_t[:], in_=iota16_d[:, :])
                nc.sync.dma_start(out=iotac_t[:], in_=iotac_d[:, :])
                nc.sync.dma_start(out=dstI_t[:], in_=dstI_d[:, :])
                nc.sync.dma_start(out=srcL_t[:], in_=srcL_d[:, :])
            fb_all = ownp.tile([P, ntiles, 2 * OUT_DIM], F16, tag="fb_all")
            s_all = ownp.tile([P, ntiles, 2], F16, tag="s_all")
            s16_all = ownp.tile([P, ntiles, C, 2], F16, tag="s16_all")
            agg_sb = ownp.tile([P, ntiles, RHSW], F32, tag="agg_sb")

            _mark("consts_end")

            # ---------------- phase 1: build whaug table ----------------
            def s_expand(t):
                srcr = srcrp.tile([P, C, P], U8, tag="srcr")
                sl = srcR_d[0:1, t * C * P : (t + 1) * C * P]
                nc.sync.dma_start(
                    out=srcr[:],
                    in_=bass.AP(sl.tensor, sl.offset, [[0, P], [1, C * P]]),
                )
                oneT = onehp.tile([P, C, P], F16, tag="oneT")
                nc.vector.tensor_tensor(
                    out=oneT[:],
                    in0=_ap_expand(iotac_t[:], [(0, C), (0, P)]),
                    in1=srcr[:],
                    op=mybir.AluOpType.is_equal,
                )
                s_ps = spsp.tile([P, C, 2], F32, tag="s_ps")
                for c in range(C):
                    nc.tensor.matmul(
                        out=s_ps[:, c, :],
                        lhsT=oneT[:, c, :],
                        rhs=s_all[:, t, :],
                        start=True,
                        stop=True,
                    )
                nc.vector.tensor_copy(out=s16_all[:, t, :, :], in_=s_ps[:])

            n0 = 0
            while n0 < n_alltiles:
                span = min(cfg.span_tiles, n_alltiles - n0)
                xk = xkp.tile([P, 2, cfg.span_tiles * P], F16, tag="xk")
                for kt in range(2):
                    nc.sync.dma_start(
                        out=xk[:, kt, 0 : span * P],
                        in_=xT_d[kt * P : (kt + 1) * P, n0 * P : (n0 + span) * P],
                    )
                g0 = 0
                while g0 < span:
                    GRP = 4
                    grp = min(GRP, span - g0)
                    aug = auggp.tile([P, 4, AUGW], F16, tag="aug")
                    for g in range(grp):
                        nt = g0 + g
                        gtile = n0 + nt
                        ps = bldps.tile([P, USEW], F32, tag="bld")
                        for kt in range(2):
                            nc.tensor.matmul(
                                out=ps[:],
                                lhsT=xk[:, kt, nt * P : (nt + 1) * P],
                                rhs=wb[:, kt, :],
                                start=(kt == 0),
                                stop=(kt == 1),
                            )
                        nc.scalar.activation(
                            out=aug[:, g, 0:USEW],
                            in_=ps[:],
                            func=mybir.ActivationFunctionType.Copy,
                        )
                        if gtile < ntiles:
                            nc.vector.tensor_copy(
                                out=fb_all[:, gtile, :], in_=ps[:, 0 : 2 * OUT_DIM]
                            )
                            nc.vector.tensor_copy(
                                out=s_all[:, gtile, :], in_=ps[:, 258:260]
                            )
                    r0 = (n0 + g0) * P
                    nc.sync.dma_start(
                        out=whaug_d[r0 : r0 + grp * P, :].rearrange(
                            "(g p) c -> p g c", p=P
                        ),
                        in_=aug[:, 0:grp, :],
                    )
                    g0 += grp
                n0 += span
                if n0 == cfg.span_tiles:
                    load_p2_consts()

            _mark("p1_end")

            # ---------------- phase 2: passes of CP chunks ----------------
            OGRP = 8

            def compute(t, p):
                c0 = bounds[p]
                c1 = bounds[p + 1]
                nch = c1 - c0
                gall = gallp.tile([P, CPmax, USEW], F16, tag="gall")
                for c in range(c0, c1):
                    R = int(rtab[t][c])
                    bound = bass.AP(
                        whaug_ref.tensor, 0, [[AUGW, R], [1, USEW]]
                    )
                    nc.gpsimd.indirect_dma_start(
                        out=gall[:, c - c0, :],
                        out_offset=None,
                        in_=bound,
                        in_offset=IndirectOffsetOnAxis(
                            ap=dstI_t[:, t * C + c : t * C + c + 1], axis=0
                        ),
                    )
                # e = lrelu(s + t); pexp = exp(e - SHIFT)
                e_t = finp.tile([P, CPmax * 2], F32, tag="e_t")
                nc.vector.tensor_tensor(
                    out=e_t[:, 0 : nch * 2],
                    in0=s16_all[:, t, c0:c1, :],
                    in1=gall[:, 0:nch, 256:258],
                    op=mybir.AluOpType.add,
                )
                e_s = finp.tile([P, CPmax * 2], F32, tag="e_s")
                nc.vector.tensor_scalar(
                    out=e_s[:, 0 : nch * 2], in0=e_t[:, 0 : nch * 2],
                    scalar1=0.2, scalar2=None, op0=mybir.AluOpType.mult,
                )
                lr_t = finp.tile([P, CPmax * 2], F32, tag="lr_t")
                nc.vector.tensor_tensor(
                    out=lr_t[:, 0 : nch * 2], in0=e_t[:, 0 : nch * 2],
                    in1=e_s[:, 0 : nch * 2], op=mybir.AluOpType.max,
                )
                p16 = finp.tile([P, CPmax, 2], F16, tag="p16")
                nc.scalar.activation(
                    out=p16[:, 0:nch, :].rearrange("p c h -> p (c h)"),
                    in_=lr_t[:, 0 : nch * 2],
                    func=mybir.ActivationFunctionType.Exp,
                    bias=shift_t[:, 0:1],
                )
                oneh = onehp.tile([P, CPmax, P], F16, tag="oneh")
                nc.vector.tensor_tensor(
                    out=oneh[:, 0:nch, :],
                    in0=_ap_expand(srcL_t[:, t * C + c0 : t * C + c1], [(1, nch), (0, P)]),
                    in1=_ap_expand(iota16_t[:], [(0, nch), (1, P)]),
                    op=mybir.AluOpType.is_equal,
                )
                rhs = rhsp.tile([P, CPmax, RHSW], F16, tag="rhs")
                nc.vector.tensor_tensor(
                    out=rhs[:, 0:nch, 0 : 2 * OUT_DIM],
                    in0=gall[:, 0:nch, 0 : 2 * OUT_DIM],
                    in1=_ap_expand(p16[:], [(2, nch), (1, 2), (0, OUT_DIM)]),
                    op=mybir.AluOpType.mult,
                )
                nc.vector.tensor_copy(
                    out=rhs[:, 0:nch, 2 * OUT_DIM : RHSW], in_=p16[:, 0:nch, :]
                )
                ps = aggps.tile([P, RHSW], F32, tag="agg")
                for c in range(nch):
                    nc.tensor.matmul(
                        out=ps[:],
                        lhsT=oneh[:, c, :],
                        rhs=rhs[:, c, :],
                        start=(c == 0),
                        stop=(c == nch - 1),
                    )
                if p == 0:
                    nc.vector.tensor_copy(out=agg_sb[:, t, :], in_=ps[:])
                else:
                    nc.vector.tensor_tensor(
                        out=agg_sb[:, t, :], in0=agg_sb[:, t, :], in1=ps[:],
                        op=mybir.AluOpType.add,
                    )

            def finalize(t, og, g):
                den2 = finp.tile([P, 2], F32, tag="den2")
                nc.vector.tensor_scalar(
                    out=den2[:], in0=agg_sb[:, t, 2 * OUT_DIM : RHSW],
                    scalar1=DELTA, scalar2=None, op0=mybir.AluOpType.add,
                )
                rcp = finp.tile([P, 2], F32, tag="rcp")
                nc.vector.reciprocal(out=rcp[:], in_=den2[:])
                num2 = finp.tile([P, 2 * OUT_DIM], F32, tag="num2")
                nc.vector.scalar_tensor_tensor(
                    out=num2[:],
                    in0=fb_all[:, t, :],
                    scalar=DELTA,
                    in1=agg_sb[:, t, 0 : 2 * OUT_DIM],
                    op0=mybir.AluOpType.mult,
                    op1=mybir.AluOpType.add,
                )
                for h in range(2):
                    nc.scalar.activation(
                        out=og[:, g, h * OUT_DIM : (h + 1) * OUT_DIM],
                        in_=num2[:, h * OUT_DIM : (h + 1) * OUT_DIM],
                        func=mybir.ActivationFunctionType.Copy,
                        scale=rcp[:, h : h + 1],
                    )

            for rep in range(cfg.reps):
                for p in range(NP_ - 1):
                    for t in range(ntiles):
                        if rep == 0 and p == 0:
                            s_expand(t)
                        compute(t, p)
                # last pass: finalize tile t-1 after computing tile t
                og = None
                pend = None

                def flush(tf, og):
                    g = tf % OGRP
                    if g == OGRP - 1 or tf == ntiles - 1:
                        t0 = tf - g
                        nc.sync.dma_start(
                            out=out_d[t0 * P : (tf + 1) * P, :].rearrange(
                                "(g p) c -> p g c", p=P
                            ),
                            in_=og[:, 0 : g + 1, :],
                        )

                for t in range(ntiles):
                    compute(t, NP_ - 1)
                    if pend is not None:
                        g = pend % OGRP
                        if g == 0:
                            og = ogp.tile([P, OGRP, 2 * OUT_DIM], F32, tag="og")
                        finalize(pend, og, g)
                        flush(pend, og)
                    pend = t
                g = pend % OGRP
                if g == 0:
                    og = ogp.tile([P, OGRP, 2 * OUT_DIM], F32, tag="og")
                finalize(pend, og, g)
                flush(pend, og)
            _mark("p2_end")

    nc.compile()
    return nc


_prog_cache = {}


def kernel(x, edge_index, W_w, W_b, a):
    cfg, shared, per_core = host_prep(x, edge_index, W_w, W_b, a, n_cores=8)
    rtab = shared["_Rtab"]
    key = (cfg, rtab.tobytes())
    if key not in _prog_cache:
        _prog_cache[key] = build_program(cfg, rtab)
    nc = _prog_cache[key]
    in_maps = [
        {kk: v for kk, v in {**shared, **pc}.items() if not kk.startswith("_")}
        for pc in per_core
    ]
    res = run_bass_kernel_spmd(nc, in_maps, list(range(cfg.n_cores)))
    out = np.zeros((cfg.n_nodes, 2 * OUT_DIM), dtype=np.float32)
    for k in range(cfg.n_cores):
        pc = per_core[k]
        out[pc["_nodes"]] = res.results[k]["out"][pc["_rows"]]
    return out


# revision 29
# speedup vs baseline: 1.5279x; 1.3388x over previous
"""GAT layer kernel for Trainium2, 8 NeuronCores.

Strategy (src-range sharding, no collectives):
  - Host: LPT-balance src nodes over 392 global (core, tile) slots so each
    tile owns ~2048 edges (C=16 chunks of 128).  Table rows live in
    per-core ROTATED tile-slot space: core k's own nodes occupy rows
    [0, ntiles*128) so phase-2 fallback/s data comes straight from SBUF.
  - Phase 1 (device): whaug table [npad, 264] fp16 rows
    [Wh0(128) | Wh1(128) | t0 t1 s0 s1 | pad2] via x @ [W | wt | ws]
    matmuls; PSUM->fp16 conversion on the Activation engine (one copy per
    tile); own-tile rows also kept in SBUF (fb_all) + s-pairs (s_all).
  - Phase 2 (device), per tile:
      Pool : C indirect row-gathers (dst rows, 128 offsets each)
      PE   : srcL broadcast (ones @ srcL_row), C tiny s-expand matmuls,
             C aggregation matmuls (one-hot lhsT)
      DVE  : one-hot builds (oneh [e,slot], oneT [slot,e]), rhs scaling by
             p, tiny e-ops; reciprocal
      ACT  : exp, output scaling
    Softmax uses a global SHIFT (softmax-invariant); deg-0 fallback uses
    out = (num + d*Wh_own) / (den + d) with d=1e-30 (exact for den=0).
"""

import math
import sys
from dataclasses import dataclass

import numpy as np

sys.path.insert(0, "/opt/trn_rl_repo")

import concourse.bass as bass
import concourse.mybir as mybir
import concourse.tile as tile
from concourse import bacc
from concourse.bass import IndirectOffsetOnAxis
from concourse.bass_utils import run_bass_kernel_spmd

N_NODES = 50000
IN_DIM = 256
OUT_DIM = 128
NUM_HEADS = 2

P = 128
AUGW = 260  # table row: Wh0(128), Wh1(128), t0, t1, s0, s1
USEW = 260
RHSW = 258  # aggregation matmul width: 256 features + 2 denominator cols
SHIFT = 4.0
DELTA = 1e-30

F32 = mybir.dt.float32
F16 = mybir.dt.float16
I32 = mybir.dt.int32


@dataclass(frozen=True)
class Cfg:
    n_nodes: int
    n_cores: int
    C: int
    span_tiles: int = 8
    reps: int = 1
    npass: int = 4
    bounds: tuple = None

    @property
    def nodes_per_core(self):
        return self.n_nodes // self.n_cores

    @property
    def ntiles(self):
        return (self.nodes_per_core + P - 1) // P

    @property
    def npad(self):
        return self.n_cores * self.ntiles * P


def _ap_expand(ap, dims):
    """Return an AP keeping ap's partition dim and replacing the free dims
    with `dims` = list of (step, count) pairs (element units)."""
    return bass.AP(ap.tensor, ap.offset, [list(ap.ap[0])] + [[s, c] for s, c in dims])


def host_prep(x, edge_index, W_w, W_b, a, n_cores=8):
    """Index/layout preprocessing + parameter folding."""
    x = np.asarray(x, dtype=np.float32)
    edge_index = np.asarray(edge_index)
    W_w = np.asarray(W_w, dtype=np.float32)
    W_b = np.asarray(W_b, dtype=np.float32)
    a = np.asarray(a, dtype=np.float32)
    assert np.abs(W_b).max() == 0.0, "nonzero bias not supported"

    n_nodes, in_dim = x.shape
    D = OUT_DIM
    n_edges = edge_index.shape[1]

    # wbig columns: [W (256) | wt0 wt1 | ws0 ws1]  ->  ps = [Wh0 Wh1 t0 t1 s0 s1]
    a_src, a_dst = a[:D], a[D:]
    ws0 = W_w[:, 0:D] @ a_src
    ws1 = W_w[:, D : 2 * D] @ a_src
    wt0 = W_w[:, 0:D] @ a_dst
    wt1 = W_w[:, D : 2 * D] @ a_dst
    wbig = np.concatenate(
        [W_w, wt0[:, None], wt1[:, None], ws0[:, None], ws1[:, None]], axis=1
    ).astype(np.float16)  # [in_dim, 260]

    src = np.asarray(edge_index[0], dtype=np.int64)
    dst = np.asarray(edge_index[1], dtype=np.int64)
    order = np.argsort(src, kind="stable")
    src_s = src[order]
    dst_s = dst[order].astype(np.int64)

    npc = n_nodes // n_cores
    ntiles = (npc + P - 1) // P

    # LPT: assign nodes to (core,tile,slot), balancing per-tile edge counts.
    import heapq

    ntile_tot = n_cores * ntiles
    deg_all = np.bincount(src, minlength=n_nodes)
    order_n = np.argsort(-deg_all, kind="stable")
    heap = [(0, t) for t in range(ntile_tot)]
    heapq.heapify(heap)
    fill = np.zeros(ntile_tot, dtype=np.int64)
    node_tile = np.zeros(n_nodes, dtype=np.int64)
    node_slot = np.zeros(n_nodes, dtype=np.int64)
    for n in order_n:
        while True:
            w, t = heapq.heappop(heap)
            if fill[t] < P:
                break
        node_tile[n] = t
        node_slot[n] = fill[t]
        fill[t] += 1
        if fill[t] < P:
            heapq.heappush(heap, (w + int(deg_all[n]), t))

    grow = node_tile * P + node_slot  # global tile-slot row of each node

    gtile = node_tile[src_s]
    pos = node_slot[src_s]
    # sort by (tile, rotated dst-row of the owning core) so chunk c of a
    # tile covers a dst-row prefix bound in that core's table write order
    # -> gathers can start before the whole table is written
    rows_pc_ = ntiles * P
    npad_ = n_cores * rows_pc_
    rot_key = (grow[dst_s] - (gtile // ntiles) * rows_pc_) % npad_
    order2 = np.lexsort((rot_key, gtile))
    gtile = gtile[order2]
    dst_s = dst_s[order2]
    pos = pos[order2]

    counts = np.bincount(gtile, minlength=ntile_tot)
    C = int(math.ceil(counts.max() / P))
    cfg = Cfg(n_nodes=n_nodes, n_cores=n_cores, C=C)
    npad = cfg.npad
    slots_per_tile = C * P

    # xT in global tile-slot space (empty slots -> 0 rows)
    xTslot = np.zeros((in_dim, npad), dtype=np.float16)
    xTslot[:, grow] = x.T.astype(np.float16)

    starts = np.zeros(ntile_tot, dtype=np.int64)
    starts[1:] = np.cumsum(counts)[:-1]
    slot_in_tile = np.arange(n_edges) - starts[gtile]

    # per-edge-slot arrays in (tile, chunk, partition) space
    dstG = np.full((ntile_tot, slots_per_tile), -1, dtype=np.int64)  # dst grow
    srcL = np.full((ntile_tot, slots_per_tile), 255, dtype=np.uint8)
    flat = gtile * slots_per_tile + slot_in_tile
    dstG.reshape(-1)[flat] = grow[dst_s]
    srcL.reshape(-1)[flat] = pos.astype(np.uint8)

    def to_core_layout(arr, fillval):
        # [tiles, C*P] -> per-core [P, ntiles*C]
        a4 = arr.reshape(n_cores, ntiles, C, P)
        return np.ascontiguousarray(np.transpose(a4, (0, 3, 1, 2))).reshape(
            n_cores, P, ntiles * C
        )

    dstG_c = to_core_layout(dstG, 0)
    srcL_c = to_core_layout(srcL, -1.0)

    # Rtab[t, c] = 1 + max over cores of the rotated dst row in chunk c of
    # tile t (pad slots have dstG=0 -> rotated row npad-shift; make pads
    # point at row 0 instead so they don't inflate the bound)

    iota16 = np.broadcast_to(
        np.arange(P, dtype=np.uint8), (P, P)
    ).copy()  # iota16[p, j] = j
    iota_col = np.arange(P, dtype=np.uint8).reshape(P, 1).copy()

    shared = {"wbig": wbig, "iota16": iota16, "iotac": iota_col}
    shared["_Rtab"] = None  # placeholder, set below
    per_core = []
    rows_pc = ntiles * P
    # srcL row layout for the PE broadcast: [1, ntiles*C*P]
    Rtab = np.zeros((ntiles, C), dtype=np.int64)
    dstI_all = []
    for k in range(n_cores):
        shift = k * rows_pc
        rot = (dstG_c[k] - shift) % npad
        rot[dstG_c[k] < 0] = 0  # pad slots -> row 0
        dstI_all.append(rot.astype(np.int32))
        r3 = rot.reshape(P, ntiles, C)
        Rtab = np.maximum(Rtab, r3.max(axis=0))
    Rtab = Rtab + 1
    for k in range(n_cores):
        shift = k * rows_pc
        xT_k = np.roll(xTslot, -shift, axis=1)
        dstI_k = dstI_all[k]
        srcL_k = srcL_c[k]
        # row layout value at (t, c, p) = srcL_k[p, t*C + c]
        srcR_k = np.ascontiguousarray(
            np.transpose(srcL_k.reshape(P, ntiles, C), (1, 2, 0))
        ).reshape(1, ntiles * C * P)
        mine = (node_tile >= k * ntiles) & (node_tile < (k + 1) * ntiles)
        nodes_k = np.nonzero(mine)[0]
        rows_k = (node_tile[nodes_k] - k * ntiles) * P + node_slot[nodes_k]
        per_core.append(
            {
                "xT": xT_k,
                "dstI": dstI_k,
                "srcL": srcL_k,
                "srcR": srcR_k,
                "_nodes": nodes_k,
                "_rows": rows_k,
            }
        )
    shared["_Rtab"] = Rtab
    return cfg, shared, per_core


def build_program(cfg: Cfg, rtab, marks=None):
    """rtab: [ntiles, C] int array; rtab[t][c] = exclusive upper bound on
    table rows referenced by chunk c of tile t (edges are dst-sorted within
    each tile, so this is a prefix bound enabling gather/build overlap)."""
    C, ntiles, npad = cfg.C, cfg.ntiles, cfg.npad
    NP_ = cfg.npass
    CP = (C + NP_ - 1) // NP_
    if cfg.bounds is not None and C == 16:
        bounds = list(cfg.bounds)
        NP_ = len(bounds) - 1
    else:
        bounds = [min(C, i * CP) for i in range(NP_ + 1)]
    CPmax = max(bounds[i + 1] - bounds[i] for i in range(NP_))

    nc = bacc.Bacc("TRN2", target_bir_lowering=False, debug=False)

    def _mark(label):
        if marks is not None:
            marks[label] = sum(len(b.instructions) for b in nc.m.functions[0].blocks)

    U8 = mybir.dt.uint8

    xT_d = nc.dram_tensor("xT", [IN_DIM, npad], F16, kind="ExternalInput")
    wbig_d = nc.dram_tensor("wbig", [IN_DIM, USEW], F16, kind="ExternalInput")
    iota16_d = nc.dram_tensor("iota16", [P, P], U8, kind="ExternalInput")
    iotac_d = nc.dram_tensor("iotac", [P, 1], U8, kind="ExternalInput")
    dstI_d = nc.dram_tensor("dstI", [P, ntiles * C], I32, kind="ExternalInput")
    srcL_d = nc.dram_tensor("srcL", [P, ntiles * C], U8, kind="ExternalInput")
    srcR_d = nc.dram_tensor("srcR", [1, ntiles * C * P], U8, kind="ExternalInput")
    out_d = nc.dram_tensor("out", [ntiles * P, 2 * OUT_DIM], F32, kind="ExternalOutput")

    whaug_d = nc.dram_tensor("whaug", [npad, AUGW], F16)
    whaug_ref = whaug_d[:, :]

    n_alltiles = npad // P

    with tile.TileContext(nc) as tc:
        with (
            tc.tile_pool(name="const", bufs=1) as constp,
            tc.tile_pool(name="xk", bufs=2) as xkp,
            tc.tile_pool(name="bld_ps", bufs=3, space="PSUM") as bldps,
            tc.tile_pool(name="augg", bufs=2) as auggp,
            tc.tile_pool(name="own", bufs=1) as ownp,
            tc.tile_pool(name="gall", bufs=max(2, 24 // CPmax)) as gallp,
            tc.tile_pool(name="srcr", bufs=2) as srcrp,
            tc.tile_pool(name="oneh", bufs=max(2, 16 // CPmax)) as onehp,
            tc.tile_pool(name="rhs", bufs=max(2, 16 // CPmax)) as rhsp,
            tc.tile_pool(name="s_ps", bufs=2, space="PSUM") as spsp,
            tc.tile_pool(name="agg_ps", bufs=3, space="PSUM") as aggps,
            tc.tile_pool(name="fin", bufs=4) as finp,
            tc.tile_pool(name="og", bufs=2) as ogp,
        ):
            # ---------------- constants ----------------
            wb = constp.tile([P, 2, USEW], F16, tag="wb")
            nc.sync.dma_start(
                out=wb[:], in_=wbig_d[:, :].rearrange("(kt kp) c -> kp kt c", kp=P)
            )
            iota16_t = constp.tile([P, P], U8, tag="iota16")
            iotac_t = constp.tile([P, 1], U8, tag="iotac")
            dstI_t = constp.tile([P, ntiles * C], I32, tag="dstI")
            srcL_t = constp.tile([P, ntiles * C], U8, tag="srcL")
            shift_t = constp.tile([P, 1], F32, tag="shift")
            nc.vector.memset(shift_t[:], -SHIFT)

            def load_p2_consts():
                nc.sync.dma_start(out=iota16_t[:], in_=iota16_d[:, :])
                nc.sync.dma_start(out=iotac_t[:], in_=iotac_d[:, :])
                nc.sync.dma_start(out=dstI_t[:], in_=dstI_d[:, :])
                nc.sync.dma_start(out=srcL_t[:], in_=srcL_d[:, :])
            fb_all = ownp.tile([P, ntiles, 2 * OUT_DIM], F16, tag="fb_all")
            s_all = ownp.tile([P, ntiles, 2], F16, tag="s_all")
            s16_all = ownp.tile([P, ntiles, C, 2], F16, tag="s16_all")
            agg_sb = ownp.tile([P, ntiles, RHSW], F32, tag="agg_sb")

            _mark("consts_end")

            # ---------------- phase 1: build whaug table ----------------
            def s_expand(t):
                srcr = srcrp.tile([P, C, P], U8, tag="srcr")
                sl = srcR_d[0:1, t * C * P : (t + 1) * C * P]
                nc.sync.dma_start(
                    out=srcr[:],
                    in_=bass.AP(sl.tensor, sl.offset, [[0, P], [1, C * P]]),
                )
                oneT = onehp.tile([P, C, P], F16, tag="oneT")
                nc.vector.tensor_tensor(
                    out=oneT[:],
                    in0=_ap_expand(iotac_t[:], [(0, C), (0, P)]),
                    in1=srcr[:],
                    op=mybir.AluOpType.is_equal,
                )
                s_ps = spsp.tile([P, C, 2], F32, tag="s_ps")
                for c in range(C):
                    nc.tensor.matmul(
                        out=s_ps[:, c, :],
                        lhsT=oneT[:, c, :],
                        rhs=s_all[:, t, :],
                        start=True,
                        stop=True,
                    )
                nc.vector.tensor_copy(out=s16_all[:, t, :, :], in_=s_ps[:])

            n0 = 0
            while n0 < n_alltiles:
                span = min(cfg.span_tiles, n_alltiles - n0)
                xk = xkp.tile([P, 2, cfg.span_tiles * P], F16, tag="xk")
                for kt in range(2):
                    nc.sync.dma_start(
                        out=xk[:, kt, 0 : span * P],
                        in_=xT_d[kt * P : (kt + 1) * P, n0 * P : (n0 + span) * P],
                    )
                g0 = 0
                while g0 < span:
                    GRP = 4
                    grp = min(GRP, span - g0)
                    aug = auggp.tile([P, 4, AUGW], F16, tag="aug")
                    for g in range(grp):
                        nt = g0 + g
                        gtile = n0 + nt
                        ps = bldps.tile([P, USEW], F32, tag="bld")
                        for kt in range(2):
                            nc.tensor.matmul(
                                out=ps[:],
                                lhsT=xk[:, kt, nt * P : (nt + 1) * P],
                                rhs=wb[:, kt, :],
                                start=(kt == 0),
                                stop=(kt == 1),
                            )
                        nc.scalar.activation(
                            out=aug[:, g, 0:USEW],
                            in_=ps[:],
                            func=mybir.ActivationFunctionType.Copy,
                        )
                        if gtile < ntiles:
                            nc.vector.tensor_copy(
                                out=fb_all[:, gtile, :], in_=ps[:, 0 : 2 * OUT_DIM]
                            )
                            nc.vector.tensor_copy(
                                out=s_all[:, gtile, :], in_=ps[:, 258:260]
                            )
                    r0 = (n0 + g0) * P
                    nc.sync.dma_start(
                        out=whaug_d[r0 : r0 + grp * P, :].rearrange(
                            "(g p) c -> p g c", p=P
                        ),
                        in_=aug[:, 0:grp, :],
                    )
                    g0 += grp
                n0 += span
                if n0 == cfg.span_tiles:
                    load_p2_consts()

            _mark("p1_end")

            # ---------------- phase 2: passes of CP chunks ----------------
            OGRP = 8

            def compute(t, p):
                c0 = bounds[p]
                c1 = bounds[p + 1]
                nch = c1 - c0
                gall = gallp.tile([P, CPmax, USEW], F16, tag="gall")
                for c in range(c0, c1):
                    R = int(rtab[t][c])
                    bound = bass.AP(
                        whaug_ref.tensor, 0, [[AUGW, R], [1, USEW]]
                    )
                    nc.gpsimd.indirect_dma_start(
                        out=gall[:, c - c0, :],
                        out_offset=None,
                        in_=bound,
                        in_offset=IndirectOffsetOnAxis(
                            ap=dstI_t[:, t * C + c : t * C + c + 1], axis=0
                        ),
                    )
                # e = lrelu(s + t); pexp = exp(e - SHIFT)
                e_t = finp.tile([P, CPmax * 2], F32, tag="e_t")
                nc.vector.tensor_tensor(
                    out=e_t[:, 0 : nch * 2],
                    in0=s16_all[:, t, c0:c1, :],
                    in1=gall[:, 0:nch, 256:258],
                    op=mybir.AluOpType.add,
                )
                e_s = finp.tile([P, CPmax * 2], F32, tag="e_s")
                nc.vector.tensor_scalar(
                    out=e_s[:, 0 : nch * 2], in0=e_t[:, 0 : nch * 2],
                    scalar1=0.2, scalar2=None, op0=mybir.AluOpType.mult,
                )
                lr_t = finp.tile([P, CPmax * 2], F32, tag="lr_t")
                nc.vector.tensor_tensor(
                    out=lr_t[:, 0 : nch * 2], in0=e_t[:, 0 : nch * 2],
                    in1=e_s[:, 0 : nch * 2], op=mybir.AluOpType.max,
                )
                p16 = finp.tile([P, CPmax, 2], F16, tag="p16")
                nc.scalar.activation(
                    out=p16[:, 0:nch, :].rearrange("p c h -> p (c h)"),
                    in_=lr_t[:, 0 : nch * 2],
                    func=mybir.ActivationFunctionType.Exp,
                    bias=shift_t[:, 0:1],
                )
                oneh = onehp.tile([P, CPmax, P], F16, tag="oneh")
                nc.vector.tensor_tensor(
                    out=oneh[:, 0:nch, :],
                    in0=_ap_expand(srcL_t[:, t * C + c0 : t * C + c1], [(1, nch), (0, P)]),
                    in1=_ap_expand(iota16_t[:], [(0, nch), (1, P)]),
                    op=mybir.AluOpType.is_equal,
                )
                rhs = rhsp.tile([P, CPmax, RHSW], F16, tag="rhs")
                nc.vector.tensor_tensor(
                    out=rhs[:, 0:nch, 0 : 2 * OUT_DIM],
                    in0=gall[:, 0:nch, 0 : 2 * OUT_DIM],
                    in1=_ap_expand(p16[:], [(2, nch), (1, 2), (0, OUT_DIM)]),
                    op=mybir.AluOpType.mult,
                )
                nc.vector.tensor_copy(
                    out=rhs[:, 0:nch, 2 * OUT_DIM : RHSW], in_=p16[:, 0:nch, :]
                )
                ps = aggps.tile([P, RHSW], F32, tag="agg")
                for c in range(nch):
                    nc.tensor.matmul(
                        out=ps[:],
                        lhsT=oneh[:, c, :],
                        rhs=rhs[:, c, :],
                        start=(c == 0),
                        stop=(c == nch - 1),
                    )
                if p == 0:
                    nc.vector.tensor_copy(out=agg_sb[:, t, :], in_=ps[:])
                else:
                    nc.vector.tensor_tensor(
                        out=agg_sb[:, t, :], in0=agg_sb[:, t, :], in1=ps[:],
                        op=mybir.AluOpType.add,
                    )

            def finalize(t, og, g):
                den2 = finp.tile([P, 2], F32, tag="den2")
                nc.vector.tensor_scalar(
                    out=den2[:], in0=agg_sb[:, t, 2 * OUT_DIM : RHSW],
                    scalar1=DELTA, scalar2=None, op0=mybir.AluOpType.add,
                )
                rcp = finp.tile([P, 2], F32, tag="rcp")
                nc.vector.reciprocal(out=rcp[:], in_=den2[:])
                num2 = finp.tile([P, 2 * OUT_DIM], F32, tag="num2")
                nc.vector.scalar_tensor_tensor(
                    out=num2[:],
                    in0=fb_all[:, t, :],
                    scalar=DELTA,
                    in1=agg_sb[:, t, 0 : 2 * OUT_DIM],
                    op0=mybir.AluOpType.mult,
                    op1=mybir.AluOpType.add,
                )
                for h in range(2):
                    nc.scalar.activation(
                        out=og[:, g, h * OUT_DIM : (h + 1) * OUT_DIM],
                        in_=num2[:, h * OUT_DIM : (h + 1) * OUT_DIM],
                        func=mybir.ActivationFunctionType.Copy,
                        scale=rcp[:, h : h + 1],
                    )

            for rep in range(cfg.reps):
                for p in range(NP_ - 1):
                    for t in range(ntiles):
                        if rep == 0 and p == 0:
                            s_expand(t)
                        compute(t, p)
                # last pass: finalize tile t-1 after computing tile t
                og = None
                pend = None

                def flush(tf, og):
                    g = tf % OGRP
                    if g == OGRP - 1 or tf == ntiles - 1:
                        t0 = tf - g
                        nc.sync.dma_start(
                            out=out_d[t0 * P : (tf + 1) * P, :].rearrange(
                                "(g p) c -> p g c", p=P
                            ),
                            in_=og[:, 0 : g + 1, :],
                        )

                for t in range(ntiles):
                    compute(t, NP_ - 1)
                    if pend is not None:
                        g = pend % OGRP
                        if g == 0:
                            og = ogp.tile([P, OGRP, 2 * OUT_DIM], F32, tag="og")
                        finalize(pend, og, g)
                        flush(pend, og)
                    pend = t
                g = pend % OGRP
                if g == 0:
                    og = ogp.tile([P, OGRP, 2 * OUT_DIM], F32, tag="og")
                finalize(pend, og, g)
                flush(pend, og)
            _mark("p2_end")

    nc.compile()
    return nc


_prog_cache = {}


def kernel(x, edge_index, W_w, W_b, a):
    cfg, shared, per_core = host_prep(x, edge_index, W_w, W_b, a, n_cores=8)
    rtab = shared["_Rtab"]
    key = (cfg, rtab.tobytes())
    if key not in _prog_cache:
        _prog_cache[key] = build_program(cfg, rtab)
    nc = _prog_cache[key]
    in_maps = [
        {kk: v for kk, v in {**shared, **pc}.items() if not kk.startswith("_")}
        for pc in per_core
    ]
    res = run_bass_kernel_spmd(nc, in_maps, list(range(cfg.n_cores)))
    out = np.zeros((cfg.n_nodes, 2 * OUT_DIM), dtype=np.float32)
    for k in range(cfg.n_cores):
        pc = per_core[k]
        out[pc["_nodes"]] = res.results[k]["out"][pc["_rows"]]
    return out
